# revision 14
# baseline (speedup 1.0000x reference)
"""GAT encoder (4x GATConv) on 8 Trainium2 NeuronCores via Bass/Tile.

Sharding: destination nodes are split into 8 contiguous blocks (30000/8 =
3750 per core); each core owns all edges incoming to its nodes.  Per layer:
  dense phase : h = x_shard @ W, al_src/al_dst = per-head <h, a>, packed into
                table rows [h | al_src | al_dst | pad].
  AllGather   : replicate the per-rank table shard -> full node table.
  edge phase  : per dst-tile (128 dsts, G*128 edge slots) batched dma_gather
                of src rows + dst aux chunks; w = exp(leaky(al_s + al_d));
                segment-sum via one-hot selector matmuls accumulated in PSUM
                (the softmax denominator rides along as extra rhs columns);
                divide + bias (+ relu / head-mean).
"""

import math
import os

import numpy as np

N_NODES = 30000
R = 8  # cores
HEADS = 4
IN_F = 256
C1 = 256  # 4 * 64
C2 = 512  # 4 * 128
C34 = 256  # [h3 4*32 | h4 4*32]
AUX = 64  # aux chunk elems (f32): [al_src(<=8) pad | al_dst(<=8) pad]
AUX_DST_OFF = 8  # al_dst offset within aux chunk
SLOPE = 0.2
EPS = 1e-16


# ----------------------------------------------------------------------------
# Host-side graph preprocessing
# ----------------------------------------------------------------------------

def preprocess_graph(edge_index: np.ndarray, n_nodes: int, n_cores: int):
    """Returns (G, per_core list of dicts with src_idx/dst_idx [NT,128,IW] i16,
    dstloc [NT,128,G] f32)."""
    src = np.concatenate([edge_index[0], np.arange(n_nodes, dtype=np.int64)])
    dst = np.concatenate([edge_index[1], np.arange(n_nodes, dtype=np.int64)])
    ns = n_nodes // n_cores
    nt = math.ceil(ns / 128)

    per_core_raw = []
    g_max = 1
    for r in range(n_cores):
        m = (dst >= r * ns) & (dst < (r + 1) * ns)
        s_r, d_r = src[m], dst[m] - r * ns
        order = np.argsort(d_r, kind="stable")
        s_r, d_r = s_r[order], d_r[order]
        tiles = []
        for t in range(nt):
            tm = (d_r // 128) == t
            tiles.append((s_r[tm], d_r[tm] - t * 128))
            g_max = max(g_max, math.ceil(max(1, len(tiles[-1][0])) / 128))
        per_core_raw.append(tiles)

    g = g_max
    iw = g * 128 // 16
    per_core = []
    for r in range(n_cores):
        src_idx = np.zeros((nt, 128, iw), np.int16)
        dst_idx = np.zeros((nt, 128, iw), np.int16)
        dstloc = np.full((nt, 128, g), -1.0, np.float32)
        for t, (s_t, dl_t) in enumerate(per_core_raw[r]):
            e = len(s_t)
            sp = np.zeros(g * 128, np.int64)
            dp = np.zeros(g * 128, np.int64)
            lp = np.full(g * 128, -1.0, np.float32)
            sp[:e] = s_t
            dp[:e] = r * ns + t * 128 + dl_t  # global dst id
            lp[:e] = dl_t
            # idx j lives at [j % 16, j // 16], replicated down 128 partitions
            src_idx[t] = np.tile(sp.reshape(iw, 16).T.astype(np.int16), (8, 1))
            dst_idx[t] = np.tile(dp.reshape(iw, 16).T.astype(np.int16), (8, 1))
            # dstloc j at [j % 128, j // 128]
            dstloc[t] = lp.reshape(g, 128).T
        per_core.append(dict(src_idx=src_idx, dst_idx=dst_idx, dstloc=dstloc))
    return g, per_core


def make_consts(inputs):
    """Constant tensors shared by all cores."""
    f32 = np.float32

    def wtile(w):  # [F, K] -> [128, F//128, K]
        f, k = w.shape
        return np.ascontiguousarray(
            w.reshape(f // 128, 128, k).transpose(1, 0, 2)
        ).astype(f32)

    def bcast(v):  # [K] -> [128, K]
        return np.tile(np.asarray(v, f32).reshape(1, -1), (128, 1))

    w34 = np.concatenate([np.asarray(inputs["Wm"]), np.asarray(inputs["Wl"])], axis=1)
    consts = {
        "W1c": wtile(np.asarray(inputs["W1"])),
        "W2c": wtile(np.asarray(inputs["W2"])),
        "W34c": wtile(w34),
        "asb1": bcast(np.asarray(inputs["as1"]).ravel()),
        "adb1": bcast(np.asarray(inputs["ad1"]).ravel()),
        "asb2": bcast(np.asarray(inputs["as2"]).ravel()),
        "adb2": bcast(np.asarray(inputs["ad2"]).ravel()),
        "asb34": bcast(
            np.concatenate(
                [np.asarray(inputs["asm"]).ravel(), np.asarray(inputs["asl"]).ravel()]
            )
        ),
        "adb34": bcast(
            np.concatenate(
                [np.asarray(inputs["adm"]).ravel(), np.asarray(inputs["adl"]).ravel()]
            )
        ),
        "bb1": bcast(np.asarray(inputs["b1"])),
        "bb2": bcast(np.asarray(inputs["b2"])),
        "bb34": bcast(
            np.concatenate([np.asarray(inputs["bm"]), np.asarray(inputs["bl"])])
        ),
        "iota": np.tile(np.arange(128, dtype=f32).reshape(1, 128), (128, 1)),
    }
    return consts


# ----------------------------------------------------------------------------
# Bass program
# ----------------------------------------------------------------------------

def build_program(ns: int, g: int, n_cores: int = R, stop_after: str = ""):
    import concourse.bass as bass
    import concourse.mybir as mybir
    import concourse.tile as tile
    from concourse import bacc

    f32 = mybir.dt.float32
    i16 = mybir.dt.int16
    nt = math.ceil(ns / 128)
    iw = g * 128 // 16
    n_all = ns * n_cores
    Alu = mybir.AluOpType
    Act = mybir.ActivationFunctionType

    nc = bacc.Bacc(
        "TRN2", target_bir_lowering=False, debug=False, num_devices=n_cores
    )

    # ---- I/O ----
    def inp(name, shape, dt=f32):
        return nc.dram_tensor(name, list(shape), dt, kind="ExternalInput").ap()

    x_in = inp("x_shard", (ns, IN_F))
    src_idx_in = inp("src_idx", (nt, 128, iw), i16)
    dst_idx_in = inp("dst_idx", (nt, 128, iw), i16)
    dstloc_in = inp("dstloc", (nt, 128, g))
    consts_shapes = dict(
        W1c=(128, 2, C1), W2c=(128, 2, C2), W34c=(128, 4, C34),
        asb1=(128, C1), adb1=(128, C1), asb2=(128, C2), adb2=(128, C2),
        asb34=(128, C34), adb34=(128, C34),
        bb1=(128, C1), bb2=(128, C2), bb34=(128, 64), iota=(128, 128),
    )
    cin = {k: inp(k, v) for k, v in consts_shapes.items()}
    z3_out = nc.dram_tensor("z3", [ns, 32], f32, kind="ExternalOutput").ap()
    z4_out = nc.dram_tensor("z4", [ns, 32], f32, kind="ExternalOutput").ap()

    # ---- internal DRAM ----
    rows = {1: C1 + AUX, 2: C2 + AUX, 3: C34 + AUX}
    tshard = {
        k: nc.dram_tensor(f"t{k}_shard", [ns, v], f32).ap() for k, v in rows.items()
    }
    tfull = {
        k: nc.dram_tensor(f"t{k}_full", [n_all, v], f32, addr_space="Shared").ap()
        for k, v in rows.items()
    }
    o1 = nc.dram_tensor("o1", [ns, C1], f32).ap()
    o2 = nc.dram_tensor("o2", [ns, C2], f32).ap()

    layers = [
        # (idx, F_in, C_tot, H', C_h, in_ap, W, F_chunks, asb, adb, bb, out)
        dict(k=1, fin=IN_F, c=C1, h=4, ch=64, w="W1c", fc=2,
             asb="asb1", adb="adb1", bb="bb1", relu=True),
        dict(k=2, fin=C1, c=C2, h=4, ch=128, w="W2c", fc=2,
             asb="asb2", adb="adb2", bb="bb2", relu=True),
        dict(k=3, fin=C2, c=C34, h=8, ch=32, w="W34c", fc=4,
             asb="asb34", adb="adb34", bb="bb34", relu=False),
    ]
    lay_in = {1: x_in, 2: o1, 3: o2}
    lay_out = {1: o1, 2: o2}

    rg = [list(range(n_cores))]

    with tile.TileContext(nc) as tc:
        with (
            tc.tile_pool(name="consts", bufs=1) as cpool,
            tc.tile_pool(name="dense", bufs=3) as dpool,
            tc.tile_pool(name="gather", bufs=2) as gpool,
            tc.tile_pool(name="small", bufs=3) as spool,
            tc.tile_pool(name="psum", bufs=2, space="PSUM") as ppool,
            tc.tile_pool(name="psumT", bufs=2, space="PSUM") as tpool,
        ):
            # resident constants
            csb = {}
            for k, v in consts_shapes.items():
                t = cpool.tile(list(v), f32, tag=f"c_{k}")
                nc.sync.dma_start(t[:], cin[k][:])
                csb[k] = t
            identity = cpool.tile([128, 128], f32, tag="ident")
            from concourse.masks import make_identity
            make_identity(nc, identity[:])

            stop = False
            for L in layers:
                if stop:
                    break
                k, fin, c, hh, ch = L["k"], L["fin"], L["c"], L["h"], L["ch"]
                row = c + AUX
                fc = L["fc"]
                w_sb = csb[L["w"]]
                # ======== dense phase ========
                for i in range(nt):
                    r0 = i * 128
                    nrow = min(128, ns - r0)
                    in_t = dpool.tile([128, fin], f32, tag="in_t")
                    if nrow < 128:
                        nc.vector.memset(in_t[:], 0.0)
                    nc.sync.dma_start(in_t[:nrow], lay_in[k][r0 : r0 + nrow])
                    xT = dpool.tile([128, fc, 128], f32, tag="xT")
                    for j in range(fc):
                        pT = tpool.tile([128, 128], f32, tag="pT")
                        nc.tensor.transpose(
                            pT[:], in_t[:, j * 128 : (j + 1) * 128], identity[:]
                        )
                        nc.scalar.activation(xT[:, j], pT[:], Act.Copy)
                    ph = ppool.tile([128, 516], f32, tag="agg")
                    for j in range(fc):
                        nc.tensor.matmul(
                            out=ph[:, :c],
                            lhsT=xT[:, j],
                            rhs=w_sb[:, j, :c],
                            start=(j == 0),
                            stop=(j == fc - 1),
                        )
                    rt = dpool.tile([128, row], f32, tag="rowt")
                    nc.scalar.activation(rt[:, :c], ph[:, :c], Act.Copy)
                    nc.vector.memset(rt[:, c : c + AUX], 0.0)
                    tmp = dpool.tile([128, c], f32, tag="altmp")
                    hv = rt[:, :c].rearrange("p (h c) -> p h c", h=hh)
                    nc.vector.tensor_tensor(
                        out=tmp[:], in0=rt[:, :c], in1=csb[L["asb"]][:, :c],
                        op=Alu.mult,
                    )
                    nc.vector.tensor_reduce(
                        out=rt[:, c : c + hh],
                        in_=tmp[:].rearrange("p (h c) -> p h c", h=hh),
                        axis=mybir.AxisListType.X,
                        op=Alu.add,
                    )
                    nc.vector.tensor_tensor(
                        out=tmp[:], in0=rt[:, :c], in1=csb[L["adb"]][:, :c],
                        op=Alu.mult,
                    )
                    nc.vector.tensor_reduce(
                        out=rt[:, c + AUX_DST_OFF : c + AUX_DST_OFF + hh],
                        in_=tmp[:].rearrange("p (h c) -> p h c", h=hh),
                        axis=mybir.AxisListType.X,
                        op=Alu.add,
                    )
                    nc.sync.dma_start(tshard[k][r0 : r0 + nrow], rt[:nrow])

                if stop_after == f"dense{k}":
                    stop = True
                    continue
                # ======== all-gather the table ========
                nc.gpsimd.collective_compute(
                    "AllGather",
                    Alu.bypass,
                    replica_groups=rg,
                    ins=[tshard[k].opt()],
                    outs=[tfull[k].opt()],
                )

                if stop_after == f"ag{k}":
                    stop = True
                    continue
                if stop_after.startswith("edge") and stop_after.endswith(str(k)):
                    stop = True
                edge_mode = stop_after[4:-1] if stop else "full"
                # ======== edge phase ========
                for t in range(nt):
                    r0 = t * 128
                    nrow = min(128, ns - r0)
                    sidx = spool.tile([128, iw], i16, tag="sidx")
                    didx = spool.tile([128, iw], i16, tag="didx")
                    dloc = spool.tile([128, g], f32, tag="dloc")
                    nc.sync.dma_start(sidx[:], src_idx_in[t])
                    nc.sync.dma_start(didx[:], dst_idx_in[t])
                    nc.sync.dma_start(dloc[:], dstloc_in[t])
                    big = gpool.tile([128, g, row], f32, tag="big")
                    aux = gpool.tile([128, g, AUX], f32, tag="aux")
                    nc.gpsimd.dma_gather(
                        big[:], tfull[k][:], sidx[:], g * 128, g * 128,
                        elem_size=row, elem_step=row, single_packet=False,
                    )
                    nc.gpsimd.dma_gather(
                        aux[:], tfull[k][:, c : c + AUX], didx[:], g * 128,
                        g * 128, elem_size=AUX, elem_step=row,
                        single_packet=False,
                    )
                    if edge_mode == "g":  # gathers only
                        continue
                    # w = exp(leaky(al_src + al_dst)), written into big's
                    # al_src slot so the selector matmul rhs picks it up.
                    wt = spool.tile([128, g, hh], f32, tag="wt")
                    wt2 = spool.tile([128, g, hh], f32, tag="wt2")
                    nc.vector.tensor_tensor(
                        out=wt[:], in0=big[:, :, c : c + hh],
                        in1=aux[:, :, AUX_DST_OFF : AUX_DST_OFF + hh], op=Alu.add,
                    )
                    nc.vector.tensor_scalar_mul(wt2[:], wt[:], SLOPE)
                    nc.vector.tensor_tensor(
                        out=wt[:], in0=wt[:], in1=wt2[:], op=Alu.max
                    )
                    nc.scalar.activation(big[:, :, c : c + hh], wt[:], Act.Exp)

                    if edge_mode == "w":  # gathers + logits only
                        continue
                    po = ppool.tile([128, 516], f32, tag="agg")
                    nw = c + hh  # rhs width incl. denominator columns
                    split = nw > 512
                    if split:
                        pstat = ppool.tile([128, 8], f32, tag="stats", name="pstat")
                    else:
                        pstat = None
                    for s in range(g):
                        eq = spool.tile([128, 128], f32, tag="eq")
                        nc.vector.tensor_tensor(
                            out=eq[:],
                            in0=dloc[:, s : s + 1].to_broadcast([128, 128]),
                            in1=csb["iota"][:],
                            op=Alu.is_equal,
                        )
                        nc.vector.tensor_tensor(
                            out=big[:, s, :c].rearrange("p (h c) -> p h c", h=hh),
                            in0=big[:, s, :c].rearrange("p (h c) -> p h c", h=hh),
                            in1=big[:, s, c : c + hh][:, :, None].to_broadcast(
                                [128, hh, ch]
                            ),
                            op=Alu.mult,
                        )
                        if not split:
                            nc.tensor.matmul(
                                out=po[:, :nw], lhsT=eq[:], rhs=big[:, s, :nw],
                                start=(s == 0), stop=(s == g - 1),
                            )
                        else:
                            nc.tensor.matmul(
                                out=po[:, :c], lhsT=eq[:], rhs=big[:, s, :c],
                                start=(s == 0), stop=(s == g - 1),
                            )
                            nc.tensor.matmul(
                                out=pstat[:, :hh], lhsT=eq[:],
                                rhs=big[:, s, c : c + hh],
                                start=(s == 0), stop=(s == g - 1),
                            )
                    if edge_mode == "m":  # stop after matmuls: dump psum
                        sink = spool.tile([128, 16], f32, tag="sink")
                        nc.scalar.activation(sink[:], po[:, :16], Act.Copy)
                        continue
                    # normalize + bias (+relu / head-mean)
                    den = spool.tile([128, hh], f32, tag="den")
                    rec = spool.tile([128, hh], f32, tag="rec")
                    mean_scale = 4.0 if k == 3 else 1.0
                    den_src = pstat[:, :hh] if split else po[:, c : c + hh]
                    nc.scalar.activation(
                        den[:], den_src, Act.Copy,
                        bias=float(EPS * mean_scale), scale=float(mean_scale),
                    )
                    nc.vector.reciprocal(rec[:], den[:])
                    osb = dpool.tile([128, c], f32, tag="osb")
                    for h in range(hh):
                        nc.scalar.activation(
                            osb[:, h * ch : (h + 1) * ch],
                            po[:, h * ch : (h + 1) * ch],
                            Act.Copy,
                            scale=rec[:, h : h + 1],
                        )
                    if k < 3:
                        nc.vector.tensor_tensor(
                            out=osb[:], in0=osb[:], in1=csb[L["bb"]][:], op=Alu.add
                        )
                        if L["relu"]:
                            nc.vector.tensor_scalar_max(osb[:], osb[:], 0.0)
                        nc.sync.dma_start(
                            lay_out[k][r0 : r0 + nrow], osb[:nrow]
                        )
                    else:
                        z34 = spool.tile([128, 64], f32, tag="z34")
                        tmp32 = spool.tile([128, 32], f32, tag="tmp32")
                        for half in range(2):
                            b0 = half * 128
                            nc.vector.tensor_tensor(
                                out=z34[:, half * 32 : half * 32 + 32],
                                in0=osb[:, b0 : b0 + 32],
                                in1=osb[:, b0 + 32 : b0 + 64], op=Alu.add,
                            )
                            nc.vector.tensor_tensor(
                                out=tmp32[:], in0=osb[:, b0 + 64 : b0 + 96],
                                in1=osb[:, b0 + 96 : b0 + 128], op=Alu.add,
                            )
                            nc.vector.tensor_tensor(
                                out=z34[:, half * 32 : half * 32 + 32],
                                in0=z34[:, half * 32 : half * 32 + 32],
                                in1=tmp32[:], op=Alu.add,
                            )
                        nc.vector.tensor_tensor(
                            out=z34[:], in0=z34[:], in1=csb["bb34"][:], op=Alu.add
                        )
                        nc.sync.dma_start(
                            z3_out[r0 : r0 + nrow], z34[:nrow, 0:32]
                        )
                        nc.sync.dma_start(
                            z4_out[r0 : r0 + nrow], z34[:nrow, 32:64]
                        )

    nc.compile()
    return nc


# ----------------------------------------------------------------------------
# Entry point
# ----------------------------------------------------------------------------

_CACHE = {}


def _get_program(ns, g):
    key = (ns, g)
    if key not in _CACHE:
        _CACHE[key] = build_program(ns, g)
    return _CACHE[key]


def kernel(**inputs):
    from concourse import bass_utils

    x = np.ascontiguousarray(np.asarray(inputs["x"], np.float32))
    ei = np.asarray(inputs["edge_index"])
    n = x.shape[0]
    ns = n // R
    g, per_core = preprocess_graph(ei, n, R)
    consts = make_consts(inputs)
    nc = _get_program(ns, g)

    in_maps = []
    for r in range(R):
        m = dict(consts)
        m["x_shard"] = x[r * ns : (r + 1) * ns]
        m.update(per_core[r])
        in_maps.append(m)

    res = bass_utils.run_bass_kernel_spmd(nc, in_maps, core_ids=list(range(R)))
    z_mean = np.concatenate([res.results[r]["z3"] for r in range(R)], axis=0)
    z_logstd = np.concatenate([res.results[r]["z4"] for r in range(R)], axis=0)
    return z_mean, z_logstd


# revision 21
# speedup vs baseline: 1.2133x; 1.2133x over previous
"""GAT encoder (4x GATConv) on 8 Trainium2 NeuronCores via Bass/Tile.

Sharding: destination nodes are split into 8 contiguous blocks (30000/8 =
3750 per core); each core owns all edges incoming to its nodes.  Per layer:
  dense phase : h = x_shard @ W, al_src/al_dst = per-head <h, a>, packed into
                table rows [h | al_src | al_dst | pad].
  AllGather   : replicate the per-rank table shard -> full node table.
  edge phase  : per dst-tile (128 dsts, G*128 edge slots) batched dma_gather
                of src rows + dst aux chunks; w = exp(leaky(al_s + al_d));
                segment-sum via one-hot selector matmuls accumulated in PSUM
                (the softmax denominator rides along as extra rhs columns);
                divide + bias (+ relu / head-mean).
"""

import math
import os

import numpy as np

N_NODES = 30000
R = 8  # cores
HEADS = 4
IN_F = 256
C1 = 256  # 4 * 64
C2 = 512  # 4 * 128
C34 = 256  # [h3 4*32 | h4 4*32]
AUX = 64  # aux chunk elems (f32): [al_src(<=8) pad | al_dst(<=8) pad]
AUX_DST_OFF = 8  # al_dst offset within aux chunk
SLOPE = 0.2
EPS = 1e-16


# ----------------------------------------------------------------------------
# Host-side graph preprocessing
# ----------------------------------------------------------------------------

def preprocess_graph(edge_index: np.ndarray, n_nodes: int, n_cores: int):
    """Returns (G, cnts[NT], per_core list of dicts with src_idx [NT,128,IW]
    i16, dstloc [NT,128,G] f32).

    cnts[t] = max real edge count of dst-tile t across cores (the uniform
    num_idxs_reg value). Idx slots [cnt_r, cnts[t]) gather row 0 (dummy),
    slots >= cnts[t] are -1 (skipped by the gather)."""
    src = np.concatenate([edge_index[0], np.arange(n_nodes, dtype=np.int64)])
    dst = np.concatenate([edge_index[1], np.arange(n_nodes, dtype=np.int64)])
    ns = n_nodes // n_cores
    nt = math.ceil(ns / 128)

    per_core_raw = []
    g_max = 1
    cnts = np.zeros(nt, np.int64)
    for r in range(n_cores):
        m = (dst >= r * ns) & (dst < (r + 1) * ns)
        s_r, d_r = src[m], dst[m] - r * ns
        order = np.argsort(d_r, kind="stable")
        s_r, d_r = s_r[order], d_r[order]
        tiles = []
        for t in range(nt):
            tm = (d_r // 128) == t
            tiles.append((s_r[tm], d_r[tm] - t * 128))
            cnts[t] = max(cnts[t], len(tiles[-1][0]))
            g_max = max(g_max, math.ceil(max(1, len(tiles[-1][0])) / 128))
        per_core_raw.append(tiles)

    g = g_max
    iw = g * 128 // 16
    per_core = []
    for r in range(n_cores):
        src_idx = np.zeros((nt, 128, iw), np.int16)
        dstloc = np.full((nt, 128, g), -1.0, np.float32)
        for t, (s_t, dl_t) in enumerate(per_core_raw[r]):
            e = len(s_t)
            sp = np.full(g * 128, -1, np.int64)
            sp[:cnts[t]] = 0
            lp = np.full(g * 128, -1.0, np.float32)
            sp[:e] = s_t
            lp[:e] = dl_t
            # idx j lives at [j % 16, j // 16], replicated down 128 partitions
            src_idx[t] = np.tile(sp.reshape(iw, 16).T.astype(np.int16), (8, 1))
            # dstloc j at [j % 128, j // 128]
            dstloc[t] = lp.reshape(g, 128).T
        per_core.append(dict(src_idx=src_idx, dstloc=dstloc))
    return g, [int(c) for c in cnts], per_core


def make_consts(inputs):
    """Constant tensors shared by all cores."""
    f32 = np.float32

    def wtile(w):  # [F, K] -> [128, F//128, K]
        f, k = w.shape
        return np.ascontiguousarray(
            w.reshape(f // 128, 128, k).transpose(1, 0, 2)
        ).astype(f32)

    def bcast(v):  # [K] -> [128, K]
        return np.tile(np.asarray(v, f32).reshape(1, -1), (128, 1))

    w34 = np.concatenate([np.asarray(inputs["Wm"]), np.asarray(inputs["Wl"])], axis=1)
    consts = {
        "W1c": wtile(np.asarray(inputs["W1"])),
        "W2c": wtile(np.asarray(inputs["W2"])),
        "W34c": wtile(w34),
        "asb1": bcast(np.asarray(inputs["as1"]).ravel()),
        "adb1": bcast(np.asarray(inputs["ad1"]).ravel()),
        "asb2": bcast(np.asarray(inputs["as2"]).ravel()),
        "adb2": bcast(np.asarray(inputs["ad2"]).ravel()),
        "asb34": bcast(
            np.concatenate(
                [np.asarray(inputs["asm"]).ravel(), np.asarray(inputs["asl"]).ravel()]
            )
        ),
        "adb34": bcast(
            np.concatenate(
                [np.asarray(inputs["adm"]).ravel(), np.asarray(inputs["adl"]).ravel()]
            )
        ),
        "bb1": bcast(np.asarray(inputs["b1"])),
        "bb2": bcast(np.asarray(inputs["b2"])),
        "bb34": bcast(
            np.concatenate([np.asarray(inputs["bm"]), np.asarray(inputs["bl"])])
        ),
        "iota": np.tile(np.arange(128, dtype=f32).reshape(1, 128), (128, 1)),
    }
    return consts


# ----------------------------------------------------------------------------
# Bass program
# ----------------------------------------------------------------------------

def build_program(ns: int, g: int, cnts=None, n_cores: int = R, stop_after: str = ""):
    import concourse.bass as bass
    import concourse.mybir as mybir
    import concourse.tile as tile
    from concourse import bacc

    f32 = mybir.dt.float32
    i16 = mybir.dt.int16
    nt = math.ceil(ns / 128)
    if cnts is None:
        cnts = [g * 128] * nt
    iw = g * 128 // 16
    n_all = ns * n_cores
    Alu = mybir.AluOpType
    Act = mybir.ActivationFunctionType

    nc = bacc.Bacc(
        "TRN2", target_bir_lowering=False, debug=False, num_devices=n_cores
    )

    # ---- I/O ----
    def inp(name, shape, dt=f32):
        return nc.dram_tensor(name, list(shape), dt, kind="ExternalInput").ap()

    x_in = inp("x_shard", (ns, IN_F))
    src_idx_in = inp("src_idx", (nt, 128, iw), i16)
    dstloc_in = inp("dstloc", (nt, 128, g))
    consts_shapes = dict(
        W1c=(128, 2, C1), W2c=(128, 2, C2), W34c=(128, 4, C34),
        asb1=(128, C1), adb1=(128, C1), asb2=(128, C2), adb2=(128, C2),
        asb34=(128, C34), adb34=(128, C34),
        bb1=(128, C1), bb2=(128, C2), bb34=(128, 64), iota=(128, 128),
    )
    cin = {k: inp(k, v) for k, v in consts_shapes.items()}
    z3_out = nc.dram_tensor("z3", [ns, 32], f32, kind="ExternalOutput").ap()
    z4_out = nc.dram_tensor("z4", [ns, 32], f32, kind="ExternalOutput").ap()

    # ---- internal DRAM ----
    rows = {1: C1 + AUX, 2: C2 + AUX, 3: C34 + AUX}
    tshard = {
        k: nc.dram_tensor(f"t{k}_shard", [ns, v], f32).ap() for k, v in rows.items()
    }
    tfull = {
        k: nc.dram_tensor(f"t{k}_full", [n_all, v], f32, addr_space="Shared").ap()
        for k, v in rows.items()
    }
    o1 = nc.dram_tensor("o1", [ns, C1], f32).ap()
    o2 = nc.dram_tensor("o2", [ns, C2], f32).ap()

    layers = [
        # (idx, F_in, C_tot, H', C_h, in_ap, W, F_chunks, asb, adb, bb, out)
        dict(k=1, fin=IN_F, c=C1, h=4, ch=64, w="W1c", fc=2,
             asb="asb1", adb="adb1", bb="bb1", relu=True),
        dict(k=2, fin=C1, c=C2, h=4, ch=128, w="W2c", fc=2,
             asb="asb2", adb="adb2", bb="bb2", relu=True),
        dict(k=3, fin=C2, c=C34, h=8, ch=32, w="W34c", fc=4,
             asb="asb34", adb="adb34", bb="bb34", relu=False),
    ]
    lay_in = {1: x_in, 2: o1, 3: o2}
    lay_out = {1: o1, 2: o2}

    rg = [list(range(n_cores))]

    with tile.TileContext(nc) as tc:
        with (
            tc.tile_pool(name="consts", bufs=1) as cpool,
            tc.tile_pool(name="dense", bufs=3) as dpool,
            tc.tile_pool(name="gather", bufs=2) as gpool,
            tc.tile_pool(name="small", bufs=3) as spool,
            tc.tile_pool(name="psum", bufs=2, space="PSUM") as ppool,
            tc.tile_pool(name="psumT", bufs=2, space="PSUM") as tpool,
        ):
            # resident constants
            csb = {}
            for k, v in consts_shapes.items():
                t = cpool.tile(list(v), f32, tag=f"c_{k}")
                nc.sync.dma_start(t[:], cin[k][:])
                csb[k] = t
            identity = cpool.tile([128, 128], f32, tag="ident")
            from concourse.masks import make_identity
            make_identity(nc, identity[:])

            stop = False
            for L in layers:
                if stop:
                    break
                k, fin, c, hh, ch = L["k"], L["fin"], L["c"], L["h"], L["ch"]
                row = c + AUX
                fc = L["fc"]
                w_sb = csb[L["w"]]
                # ======== dense phase ========
                for i in range(nt):
                    r0 = i * 128
                    nrow = min(128, ns - r0)
                    in_t = dpool.tile([128, fin], f32, tag="in_t")
                    if nrow < 128:
                        nc.vector.memset(in_t[:], 0.0)
                    nc.sync.dma_start(in_t[:nrow], lay_in[k][r0 : r0 + nrow])
                    xT = dpool.tile([128, fc, 128], f32, tag="xT")
                    for j in range(fc):
                        pT = tpool.tile([128, 128], f32, tag="pT")
                        nc.tensor.transpose(
                            pT[:], in_t[:, j * 128 : (j + 1) * 128], identity[:]
                        )
                        nc.scalar.activation(xT[:, j], pT[:], Act.Copy)
                    ph = ppool.tile([128, 516], f32, tag="agg")
                    for j in range(fc):
                        nc.tensor.matmul(
                            out=ph[:, :c],
                            lhsT=xT[:, j],
                            rhs=w_sb[:, j, :c],
                            start=(j == 0),
                            stop=(j == fc - 1),
                        )
                    rt = dpool.tile([128, row], f32, tag="rowt")
                    nc.scalar.activation(rt[:, :c], ph[:, :c], Act.Copy)
                    nc.vector.memset(rt[:, c : c + AUX], 0.0)
                    tmp = dpool.tile([128, c], f32, tag="altmp")
                    hv = rt[:, :c].rearrange("p (h c) -> p h c", h=hh)
                    nc.vector.tensor_tensor(
                        out=tmp[:], in0=rt[:, :c], in1=csb[L["asb"]][:, :c],
                        op=Alu.mult,
                    )
                    nc.vector.tensor_reduce(
                        out=rt[:, c : c + hh],
                        in_=tmp[:].rearrange("p (h c) -> p h c", h=hh),
                        axis=mybir.AxisListType.X,
                        op=Alu.add,
                    )
                    nc.vector.tensor_tensor(
                        out=tmp[:], in0=rt[:, :c], in1=csb[L["adb"]][:, :c],
                        op=Alu.mult,
                    )
                    nc.vector.tensor_reduce(
                        out=rt[:, c + AUX_DST_OFF : c + AUX_DST_OFF + hh],
                        in_=tmp[:].rearrange("p (h c) -> p h c", h=hh),
                        axis=mybir.AxisListType.X,
                        op=Alu.add,
                    )
                    nc.sync.dma_start(tshard[k][r0 : r0 + nrow], rt[:nrow])

                if stop_after == f"dense{k}":
                    stop = True
                    continue
                # ======== all-gather the table ========
                nc.gpsimd.collective_compute(
                    "AllGather",
                    Alu.bypass,
                    replica_groups=rg,
                    ins=[tshard[k].opt()],
                    outs=[tfull[k].opt()],
                )

                if stop_after == f"ag{k}":
                    stop = True
                    continue
                if stop_after.startswith("edge") and stop_after.endswith(str(k)):
                    stop = True
                edge_mode = stop_after[4:-1] if stop else "full"
                # ======== edge phase ========
                for t in range(nt):
                    r0 = t * 128
                    nrow = min(128, ns - r0)
                    sidx = spool.tile([128, iw], i16, tag="sidx")
                    dloc = spool.tile([128, g], f32, tag="dloc")
                    nc.sync.dma_start(sidx[:], src_idx_in[t])
                    nc.sync.dma_start(dloc[:], dstloc_in[t])
                    # per-dst al_dst values of this tile's 128 dsts (local rows)
                    aldt = spool.tile([128, 8], f32, tag="aldt")
                    nc.vector.memset(aldt[:], 0.0)
                    nc.sync.dma_start(
                        aldt[:nrow, :hh],
                        tshard[k][r0 : r0 + nrow,
                                  c + AUX_DST_OFF : c + AUX_DST_OFF + hh],
                    )
                    big = gpool.tile([128, g, row], f32, tag="big")
                    if t < 2:
                        # first use of the pool slot at this layer's row
                        # width (slots are shared across layers): clear so
                        # pad-edge lanes never feed stale/NaN bit patterns
                        # into the selector matmul
                        nc.vector.memset(big[:], 0.0)
                    # clear the al_src slots: idx=-1 lanes keep 0 -> w=1,
                    # killed by their zero eq column
                    nc.vector.memset(big[:, :, c : c + hh], 0.0)
                    nc.gpsimd.dma_gather(
                        big[:], tfull[k][:], sidx[:], g * 128, cnts[t],
                        elem_size=row, elem_step=row, single_packet=False,
                    )
                    if edge_mode == "g":  # gathers only
                        continue
                    # build all G selector tiles; expand al_dst per edge via
                    # the transposed selector
                    eqall = gpool.tile([128, g, 128], f32, tag="eqall")
                    wt = spool.tile([128, g, hh], f32, tag="wt")
                    for s in range(g):
                        nc.vector.tensor_tensor(
                            out=eqall[:, s],
                            in0=dloc[:, s : s + 1].to_broadcast([128, 128]),
                            in1=csb["iota"][:],
                            op=Alu.is_equal,
                        )
                        pT2 = tpool.tile([128, 128], f32, tag="pT")
                        nc.tensor.transpose(pT2[:], eqall[:, s], identity[:])
                        eqT = spool.tile([128, 128], f32, tag="eqT")
                        nc.scalar.activation(eqT[:], pT2[:], Act.Copy)
                        pal = ppool.tile([128, 8], f32, tag="stats", name="pal")
                        nc.tensor.matmul(
                            out=pal[:, :hh], lhsT=eqT[:], rhs=aldt[:, :hh],
                            start=True, stop=True,
                        )
                        nc.scalar.activation(wt[:, s], pal[:, :hh], Act.Copy)
                    # w = exp(leaky(al_src + al_dst)), written into big's
                    # al_src slot so the selector matmul rhs picks it up.
                    wt2 = spool.tile([128, g, hh], f32, tag="wt2")
                    nc.vector.tensor_tensor(
                        out=wt[:], in0=wt[:], in1=big[:, :, c : c + hh],
                        op=Alu.add,
                    )
                    nc.vector.tensor_scalar_mul(wt2[:], wt[:], SLOPE)
                    nc.vector.tensor_tensor(
                        out=wt[:], in0=wt[:], in1=wt2[:], op=Alu.max
                    )
                    nc.scalar.activation(big[:, :, c : c + hh], wt[:], Act.Exp)

                    if edge_mode == "w":  # gathers + logits only
                        continue
                    po = ppool.tile([128, 516], f32, tag="agg")
                    nw = c + hh  # rhs width incl. denominator columns
                    split = nw > 512
                    if split:
                        pstat = ppool.tile([128, 8], f32, tag="stats", name="pstat")
                    else:
                        pstat = None
                    hd = hh // 2  # heads 0..hd-1 scaled on DVE, rest on ACT
                    for s in range(g):
                        nc.vector.tensor_tensor(
                            out=big[:, s, : hd * ch].rearrange(
                                "p (h c) -> p h c", h=hd
                            ),
                            in0=big[:, s, : hd * ch].rearrange(
                                "p (h c) -> p h c", h=hd
                            ),
                            in1=big[:, s, c : c + hd][:, :, None].to_broadcast(
                                [128, hd, ch]
                            ),
                            op=Alu.mult,
                        )
                        for h in range(hd, hh):
                            nc.scalar.activation(
                                big[:, s, h * ch : (h + 1) * ch],
                                big[:, s, h * ch : (h + 1) * ch],
                                Act.Copy,
                                scale=big[:, s, c + h : c + h + 1],
                            )
                        if not split:
                            nc.tensor.matmul(
                                out=po[:, :nw], lhsT=eqall[:, s],
                                rhs=big[:, s, :nw],
                                start=(s == 0), stop=(s == g - 1),
                            )
                        else:
                            nc.tensor.matmul(
                                out=po[:, :c], lhsT=eqall[:, s],
                                rhs=big[:, s, :c],
                                start=(s == 0), stop=(s == g - 1),
                            )
                            nc.tensor.matmul(
                                out=pstat[:, :hh], lhsT=eqall[:, s],
                                rhs=big[:, s, c : c + hh],
                                start=(s == 0), stop=(s == g - 1),
                            )
                    if edge_mode == "m":  # stop after matmuls: dump psum
                        sink = spool.tile([128, 16], f32, tag="sink")
                        nc.scalar.activation(sink[:], po[:, :16], Act.Copy)
                        continue
                    # normalize + bias (+relu / head-mean)
                    den = spool.tile([128, hh], f32, tag="den")
                    rec = spool.tile([128, hh], f32, tag="rec")
                    mean_scale = 4.0 if k == 3 else 1.0
                    den_src = pstat[:, :hh] if split else po[:, c : c + hh]
                    nc.scalar.activation(
                        den[:], den_src, Act.Copy,
                        bias=float(EPS * mean_scale), scale=float(mean_scale),
                    )
                    nc.vector.reciprocal(rec[:], den[:])
                    osb = dpool.tile([128, c], f32, tag="osb")
                    for h in range(hh):
                        nc.scalar.activation(
                            osb[:, h * ch : (h + 1) * ch],
                            po[:, h * ch : (h + 1) * ch],
                            Act.Copy,
                            scale=rec[:, h : h + 1],
                        )
                    if k < 3:
                        nc.vector.tensor_tensor(
                            out=osb[:], in0=osb[:], in1=csb[L["bb"]][:], op=Alu.add
                        )
                        if L["relu"]:
                            nc.vector.tensor_scalar_max(osb[:], osb[:], 0.0)
                        nc.sync.dma_start(
                            lay_out[k][r0 : r0 + nrow], osb[:nrow]
                        )
                    else:
                        z34 = spool.tile([128, 64], f32, tag="z34")
                        tmp32 = spool.tile([128, 32], f32, tag="tmp32")
                        for half in range(2):
                            b0 = half * 128
                            nc.vector.tensor_tensor(
                                out=z34[:, half * 32 : half * 32 + 32],
                                in0=osb[:, b0 : b0 + 32],
                                in1=osb[:, b0 + 32 : b0 + 64], op=Alu.add,
                            )
                            nc.vector.tensor_tensor(
                                out=tmp32[:], in0=osb[:, b0 + 64 : b0 + 96],
                                in1=osb[:, b0 + 96 : b0 + 128], op=Alu.add,
                            )
                            nc.vector.tensor_tensor(
                                out=z34[:, half * 32 : half * 32 + 32],
                                in0=z34[:, half * 32 : half * 32 + 32],
                                in1=tmp32[:], op=Alu.add,
                            )
                        nc.vector.tensor_tensor(
                            out=z34[:], in0=z34[:], in1=csb["bb34"][:], op=Alu.add
                        )
                        nc.sync.dma_start(
                            z3_out[r0 : r0 + nrow], z34[:nrow, 0:32]
                        )
                        nc.sync.dma_start(
                            z4_out[r0 : r0 + nrow], z34[:nrow, 32:64]
                        )

    nc.compile()
    return nc


# ----------------------------------------------------------------------------
# Entry point
# ----------------------------------------------------------------------------

_CACHE = {}


def _get_program(ns, g, cnts):
    key = (ns, g, tuple(cnts))
    if key not in _CACHE:
        _CACHE[key] = build_program(ns, g, cnts)
    return _CACHE[key]


def kernel(**inputs):
    from concourse import bass_utils

    x = np.ascontiguousarray(np.asarray(inputs["x"], np.float32))
    ei = np.asarray(inputs["edge_index"])
    n = x.shape[0]
    ns = n // R
    g, cnts, per_core = preprocess_graph(ei, n, R)
    consts = make_consts(inputs)
    nc = _get_program(ns, g, cnts)

    in_maps = []
    for r in range(R):
        m = dict(consts)
        m["x_shard"] = x[r * ns : (r + 1) * ns]
        m.update(per_core[r])
        in_maps.append(m)

    res = bass_utils.run_bass_kernel_spmd(nc, in_maps, core_ids=list(range(R)))
    z_mean = np.concatenate([res.results[r]["z3"] for r in range(R)], axis=0)
    z_logstd = np.concatenate([res.results[r]["z4"] for r in range(R)], axis=0)
    return z_mean, z_logstd


# revision 23
# speedup vs baseline: 1.2880x; 1.0615x over previous
"""GAT encoder (4x GATConv) on 8 Trainium2 NeuronCores via Bass/Tile.

Sharding: destination nodes are split into 8 contiguous blocks (30000/8 =
3750 per core); each core owns all edges incoming to its nodes.  Per layer:
  dense phase : h = x_shard @ W, al_src/al_dst = per-head <h, a>, packed into
                table rows [h | al_src | al_dst | pad].
  AllGather   : replicate the per-rank table shard -> full node table.
  edge phase  : per dst-tile (128 dsts, G*128 edge slots) batched dma_gather
                of src rows + dst aux chunks; w = exp(leaky(al_s + al_d));
                segment-sum via one-hot selector matmuls accumulated in PSUM
                (the softmax denominator rides along as extra rhs columns);
                divide + bias (+ relu / head-mean).
"""

import math
import os

import numpy as np

N_NODES = 30000
R = 8  # cores
HEADS = 4
IN_F = 256
C1 = 256  # 4 * 64
C2 = 512  # 4 * 128
C34 = 256  # [h3 4*32 | h4 4*32]
AUX = 64  # aux chunk elems (f32): [al_src(<=8) pad | al_dst(<=8) pad]
AUX_DST_OFF = 8  # al_dst offset within aux chunk
SLOPE = 0.2
EPS = 1e-16


# ----------------------------------------------------------------------------
# Host-side graph preprocessing
# ----------------------------------------------------------------------------

def preprocess_graph(edge_index: np.ndarray, n_nodes: int, n_cores: int):
    """Returns (G, cnts[NT], per_core list of dicts with src_idx [NT,128,IW]
    i16, dstloc [NT,128,G] f32).

    cnts[t] = max real edge count of dst-tile t across cores (the uniform
    num_idxs_reg value). Idx slots [cnt_r, cnts[t]) gather row 0 (dummy),
    slots >= cnts[t] are -1 (skipped by the gather)."""
    src = np.concatenate([edge_index[0], np.arange(n_nodes, dtype=np.int64)])
    dst = np.concatenate([edge_index[1], np.arange(n_nodes, dtype=np.int64)])
    ns = n_nodes // n_cores
    nt = math.ceil(ns / 128)

    per_core_raw = []
    g_max = 1
    cnts = np.zeros(nt, np.int64)
    for r in range(n_cores):
        m = (dst >= r * ns) & (dst < (r + 1) * ns)
        s_r, d_r = src[m], dst[m] - r * ns
        order = np.argsort(d_r, kind="stable")
        s_r, d_r = s_r[order], d_r[order]
        tiles = []
        for t in range(nt):
            tm = (d_r // 128) == t
            tiles.append((s_r[tm], d_r[tm] - t * 128))
            cnts[t] = max(cnts[t], len(tiles[-1][0]))
            g_max = max(g_max, math.ceil(max(1, len(tiles[-1][0])) / 128))
        per_core_raw.append(tiles)

    g = g_max
    iw = g * 128 // 16
    per_core = []
    for r in range(n_cores):
        src_idx = np.zeros((nt, 128, iw), np.int16)
        dstloc = np.full((nt, 128, g), -1.0, np.float32)
        for t, (s_t, dl_t) in enumerate(per_core_raw[r]):
            e = len(s_t)
            sp = np.full(g * 128, -1, np.int64)
            sp[:cnts[t]] = 0
            lp = np.full(g * 128, -1.0, np.float32)
            sp[:e] = s_t
            lp[:e] = dl_t
            # idx j lives at [j % 16, j // 16], replicated down 128 partitions
            src_idx[t] = np.tile(sp.reshape(iw, 16).T.astype(np.int16), (8, 1))
            # dstloc j at [j % 128, j // 128]
            dstloc[t] = lp.reshape(g, 128).T
        per_core.append(dict(src_idx=src_idx, dstloc=dstloc))
    return g, [int(c) for c in cnts], per_core


def make_consts(inputs):
    """Constant tensors shared by all cores."""
    f32 = np.float32

    def wtile(w):  # [F, K] -> [128, F//128, K]
        f, k = w.shape
        return np.ascontiguousarray(
            w.reshape(f // 128, 128, k).transpose(1, 0, 2)
        ).astype(f32)

    def bcast(v):  # [K] -> [128, K]
        return np.tile(np.asarray(v, f32).reshape(1, -1), (128, 1))

    w34 = np.concatenate([np.asarray(inputs["Wm"]), np.asarray(inputs["Wl"])], axis=1)
    consts = {
        "W1c": wtile(np.asarray(inputs["W1"])),
        "W2c": wtile(np.asarray(inputs["W2"])),
        "W34c": wtile(w34),
        "asb1": bcast(np.asarray(inputs["as1"]).ravel()),
        "adb1": bcast(np.asarray(inputs["ad1"]).ravel()),
        "asb2": bcast(np.asarray(inputs["as2"]).ravel()),
        "adb2": bcast(np.asarray(inputs["ad2"]).ravel()),
        "asb34": bcast(
            np.concatenate(
                [np.asarray(inputs["asm"]).ravel(), np.asarray(inputs["asl"]).ravel()]
            )
        ),
        "adb34": bcast(
            np.concatenate(
                [np.asarray(inputs["adm"]).ravel(), np.asarray(inputs["adl"]).ravel()]
            )
        ),
        "bb1": bcast(np.asarray(inputs["b1"])),
        "bb2": bcast(np.asarray(inputs["b2"])),
        "bb34": bcast(
            np.concatenate([np.asarray(inputs["bm"]), np.asarray(inputs["bl"])])
        ),
        "iota": np.tile(np.arange(128, dtype=f32).reshape(1, 128), (128, 1)),
    }
    return consts


# ----------------------------------------------------------------------------
# Bass program
# ----------------------------------------------------------------------------

def build_program(ns: int, g: int, cnts=None, n_cores: int = R, stop_after: str = ""):
    import concourse.bass as bass
    import concourse.mybir as mybir
    import concourse.tile as tile
    from concourse import bacc

    f32 = mybir.dt.float32
    i16 = mybir.dt.int16
    nt = math.ceil(ns / 128)
    if cnts is None:
        cnts = [g * 128] * nt
    iw = g * 128 // 16
    n_all = ns * n_cores
    Alu = mybir.AluOpType
    Act = mybir.ActivationFunctionType

    nc = bacc.Bacc(
        "TRN2", target_bir_lowering=False, debug=False, num_devices=n_cores
    )

    # ---- I/O ----
    def inp(name, shape, dt=f32):
        return nc.dram_tensor(name, list(shape), dt, kind="ExternalInput").ap()

    x_in = inp("x_shard", (ns, IN_F))
    src_idx_in = inp("src_idx", (nt, 128, iw), i16)
    dstloc_in = inp("dstloc", (nt, 128, g))
    consts_shapes = dict(
        W1c=(128, 2, C1), W2c=(128, 2, C2), W34c=(128, 4, C34),
        asb1=(128, C1), adb1=(128, C1), asb2=(128, C2), adb2=(128, C2),
        asb34=(128, C34), adb34=(128, C34),
        bb1=(128, C1), bb2=(128, C2), bb34=(128, 64), iota=(128, 128),
    )
    cin = {k: inp(k, v) for k, v in consts_shapes.items()}
    z3_out = nc.dram_tensor("z3", [ns, 32], f32, kind="ExternalOutput").ap()
    z4_out = nc.dram_tensor("z4", [ns, 32], f32, kind="ExternalOutput").ap()

    # ---- internal DRAM ----
    rows = {1: C1 + AUX, 2: C2 + AUX, 3: C34 + AUX}
    tshard = {
        k: nc.dram_tensor(f"t{k}_shard", [ns, v], f32).ap() for k, v in rows.items()
    }
    tfull = {
        k: nc.dram_tensor(f"t{k}_full", [n_all, v], f32, addr_space="Shared").ap()
        for k, v in rows.items()
    }
    o1 = nc.dram_tensor("o1", [ns, C1], f32).ap()
    o2 = nc.dram_tensor("o2", [ns, C2], f32).ap()

    layers = [
        # (idx, F_in, C_tot, H', C_h, in_ap, W, F_chunks, asb, adb, bb, out)
        dict(k=1, fin=IN_F, c=C1, h=4, ch=64, w="W1c", fc=2,
             asb="asb1", adb="adb1", bb="bb1", relu=True),
        dict(k=2, fin=C1, c=C2, h=4, ch=128, w="W2c", fc=2,
             asb="asb2", adb="adb2", bb="bb2", relu=True),
        dict(k=3, fin=C2, c=C34, h=8, ch=32, w="W34c", fc=4,
             asb="asb34", adb="adb34", bb="bb34", relu=False),
    ]
    lay_in = {1: x_in, 2: o1, 3: o2}
    lay_out = {1: o1, 2: o2}

    rg = [list(range(n_cores))]

    with tile.TileContext(nc) as tc:
        with (
            tc.tile_pool(name="consts", bufs=1) as cpool,
            tc.tile_pool(name="dense", bufs=3) as dpool,
            tc.tile_pool(name="gather", bufs=2) as gpool,
            tc.tile_pool(name="small", bufs=3) as spool,
            tc.tile_pool(name="psum", bufs=2, space="PSUM") as ppool,
            tc.tile_pool(name="psumT", bufs=2, space="PSUM") as tpool,
        ):
            # resident constants
            csb = {}
            for k, v in consts_shapes.items():
                t = cpool.tile(list(v), f32, tag=f"c_{k}")
                nc.sync.dma_start(t[:], cin[k][:])
                csb[k] = t
            identity = cpool.tile([128, 128], f32, tag="ident")
            from concourse.masks import make_identity
            make_identity(nc, identity[:])

            stop = False
            for L in layers:
                if stop:
                    break
                k, fin, c, hh, ch = L["k"], L["fin"], L["c"], L["h"], L["ch"]
                row = c + AUX
                fc = L["fc"]
                w_sb = csb[L["w"]]
                # ======== dense phase ========
                for i in range(nt):
                    r0 = i * 128
                    nrow = min(128, ns - r0)
                    in_t = dpool.tile([128, fin], f32, tag="in_t")
                    if nrow < 128:
                        nc.vector.memset(in_t[:], 0.0)
                    nc.sync.dma_start(in_t[:nrow], lay_in[k][r0 : r0 + nrow])
                    xT = dpool.tile([128, fc, 128], f32, tag="xT")
                    for j in range(fc):
                        pT = tpool.tile([128, 128], f32, tag="pT")
                        nc.tensor.transpose(
                            pT[:], in_t[:, j * 128 : (j + 1) * 128], identity[:]
                        )
                        nc.scalar.activation(xT[:, j], pT[:], Act.Copy)
                    ph = ppool.tile([128, 516], f32, tag="agg")
                    for j in range(fc):
                        nc.tensor.matmul(
                            out=ph[:, :c],
                            lhsT=xT[:, j],
                            rhs=w_sb[:, j, :c],
                            start=(j == 0),
                            stop=(j == fc - 1),
                        )
                    rt = dpool.tile([128, row], f32, tag="rowt")
                    nc.scalar.activation(rt[:, :c], ph[:, :c], Act.Copy)
                    nc.vector.memset(rt[:, c : c + AUX], 0.0)
                    tmp = dpool.tile([128, c], f32, tag="altmp")
                    hv = rt[:, :c].rearrange("p (h c) -> p h c", h=hh)
                    nc.vector.tensor_tensor(
                        out=tmp[:], in0=rt[:, :c], in1=csb[L["asb"]][:, :c],
                        op=Alu.mult,
                    )
                    nc.vector.tensor_reduce(
                        out=rt[:, c : c + hh],
                        in_=tmp[:].rearrange("p (h c) -> p h c", h=hh),
                        axis=mybir.AxisListType.X,
                        op=Alu.add,
                    )
                    nc.vector.tensor_tensor(
                        out=tmp[:], in0=rt[:, :c], in1=csb[L["adb"]][:, :c],
                        op=Alu.mult,
                    )
                    nc.vector.tensor_reduce(
                        out=rt[:, c + AUX_DST_OFF : c + AUX_DST_OFF + hh],
                        in_=tmp[:].rearrange("p (h c) -> p h c", h=hh),
                        axis=mybir.AxisListType.X,
                        op=Alu.add,
                    )
                    nc.sync.dma_start(tshard[k][r0 : r0 + nrow], rt[:nrow])

                if stop_after == f"dense{k}":
                    stop = True
                    continue
                # ======== all-gather the table ========
                nc.gpsimd.collective_compute(
                    "AllGather",
                    Alu.bypass,
                    replica_groups=rg,
                    ins=[tshard[k].opt()],
                    outs=[tfull[k].opt()],
                )

                if stop_after == f"ag{k}":
                    stop = True
                    continue
                if stop_after.startswith("edge") and stop_after.endswith(str(k)):
                    stop = True
                edge_mode = stop_after[4:-1] if stop else "full"
                # ======== edge phase ========
                for t in range(nt):
                    r0 = t * 128
                    nrow = min(128, ns - r0)
                    sidx = spool.tile([128, iw], i16, tag="sidx")
                    dloc = spool.tile([128, g], f32, tag="dloc")
                    nc.sync.dma_start(sidx[:], src_idx_in[t])
                    nc.sync.dma_start(dloc[:], dstloc_in[t])
                    # per-dst al_dst values of this tile's 128 dsts (local rows)
                    aldt = spool.tile([128, 8], f32, tag="aldt")
                    nc.vector.memset(aldt[:], 0.0)
                    nc.sync.dma_start(
                        aldt[:nrow, :hh],
                        tshard[k][r0 : r0 + nrow,
                                  c + AUX_DST_OFF : c + AUX_DST_OFF + hh],
                    )
                    big = gpool.tile([128, g, row], f32, tag="big")
                    if t < 2:
                        # first use of the pool slot at this layer's row
                        # width (slots are shared across layers): clear so
                        # pad-edge lanes never feed stale/NaN bit patterns
                        # into the selector matmul
                        nc.vector.memset(big[:], 0.0)
                    # clear the al_src slots: idx=-1 lanes keep 0 -> w=1,
                    # killed by their zero eq column
                    nc.vector.memset(big[:, :, c : c + hh], 0.0)
                    nc.gpsimd.dma_gather(
                        big[:], tfull[k][:], sidx[:], g * 128, cnts[t],
                        elem_size=row, elem_step=row, single_packet=False,
                    )
                    if edge_mode == "g":  # gathers only
                        continue
                    # build all G selector tiles; expand al_dst per edge via
                    # the transposed selector
                    eqall = gpool.tile([128, g, 128], f32, tag="eqall")
                    wt = spool.tile([128, g, hh], f32, tag="wt")
                    for s in range(g):
                        nc.vector.tensor_tensor(
                            out=eqall[:, s],
                            in0=dloc[:, s : s + 1].to_broadcast([128, 128]),
                            in1=csb["iota"][:],
                            op=Alu.is_equal,
                        )
                        pT2 = tpool.tile([128, 136], f32, tag="pT")
                        nc.tensor.transpose(
                            pT2[:, :128], eqall[:, s], identity[:]
                        )
                        eqT = spool.tile([128, 128], f32, tag="eqT")
                        nc.vector.tensor_copy(eqT[:], pT2[:, :128])
                        nc.tensor.matmul(
                            out=pT2[:, 128 : 128 + hh], lhsT=eqT[:],
                            rhs=aldt[:, :hh], start=True, stop=True,
                        )
                        nc.scalar.activation(
                            wt[:, s], pT2[:, 128 : 128 + hh], Act.Copy
                        )
                    # w = exp(leaky(al_src + al_dst)), written into big's
                    # al_src slot so the selector matmul rhs picks it up.
                    wt2 = spool.tile([128, g, hh], f32, tag="wt2")
                    nc.vector.tensor_tensor(
                        out=wt[:], in0=wt[:], in1=big[:, :, c : c + hh],
                        op=Alu.add,
                    )
                    nc.vector.tensor_scalar_mul(wt2[:], wt[:], SLOPE)
                    nc.vector.tensor_tensor(
                        out=wt[:], in0=wt[:], in1=wt2[:], op=Alu.max
                    )
                    nc.scalar.activation(big[:, :, c : c + hh], wt[:], Act.Exp)

                    if edge_mode == "w":  # gathers + logits only
                        continue
                    po = ppool.tile([128, 516], f32, tag="agg")
                    nw = c + hh  # rhs width incl. denominator columns
                    split = nw > 512
                    if split:
                        pstat = ppool.tile([128, 8], f32, tag="stats", name="pstat")
                    else:
                        pstat = None
                    for s in range(g):
                        nc.vector.tensor_tensor(
                            out=big[:, s, :c].rearrange("p (h c) -> p h c", h=hh),
                            in0=big[:, s, :c].rearrange("p (h c) -> p h c", h=hh),
                            in1=big[:, s, c : c + hh][:, :, None].to_broadcast(
                                [128, hh, ch]
                            ),
                            op=Alu.mult,
                        )
                        if not split:
                            nc.tensor.matmul(
                                out=po[:, :nw], lhsT=eqall[:, s],
                                rhs=big[:, s, :nw],
                                start=(s == 0), stop=(s == g - 1),
                            )
                        else:
                            nc.tensor.matmul(
                                out=po[:, :c], lhsT=eqall[:, s],
                                rhs=big[:, s, :c],
                                start=(s == 0), stop=(s == g - 1),
                            )
                            nc.tensor.matmul(
                                out=pstat[:, :hh], lhsT=eqall[:, s],
                                rhs=big[:, s, c : c + hh],
                                start=(s == 0), stop=(s == g - 1),
                            )
                    if edge_mode == "m":  # stop after matmuls: dump psum
                        sink = spool.tile([128, 16], f32, tag="sink")
                        nc.scalar.activation(sink[:], po[:, :16], Act.Copy)
                        continue
                    # normalize + bias (+relu / head-mean)
                    den = spool.tile([128, hh], f32, tag="den")
                    rec = spool.tile([128, hh], f32, tag="rec")
                    mean_scale = 4.0 if k == 3 else 1.0
                    den_src = pstat[:, :hh] if split else po[:, c : c + hh]
                    nc.scalar.activation(
                        den[:], den_src, Act.Copy,
                        bias=float(EPS * mean_scale), scale=float(mean_scale),
                    )
                    nc.vector.reciprocal(rec[:], den[:])
                    osb = dpool.tile([128, c], f32, tag="osb")
                    for h in range(hh):
                        nc.scalar.activation(
                            osb[:, h * ch : (h + 1) * ch],
                            po[:, h * ch : (h + 1) * ch],
                            Act.Copy,
                            scale=rec[:, h : h + 1],
                        )
                    if k < 3:
                        nc.vector.tensor_tensor(
                            out=osb[:], in0=osb[:], in1=csb[L["bb"]][:], op=Alu.add
                        )
                        if L["relu"]:
                            nc.vector.tensor_scalar_max(osb[:], osb[:], 0.0)
                        nc.sync.dma_start(
                            lay_out[k][r0 : r0 + nrow], osb[:nrow]
                        )
                    else:
                        z34 = spool.tile([128, 64], f32, tag="z34")
                        tmp32 = spool.tile([128, 32], f32, tag="tmp32")
                        for half in range(2):
                            b0 = half * 128
                            nc.vector.tensor_tensor(
                                out=z34[:, half * 32 : half * 32 + 32],
                                in0=osb[:, b0 : b0 + 32],
                                in1=osb[:, b0 + 32 : b0 + 64], op=Alu.add,
                            )
                            nc.vector.tensor_tensor(
                                out=tmp32[:], in0=osb[:, b0 + 64 : b0 + 96],
                                in1=osb[:, b0 + 96 : b0 + 128], op=Alu.add,
                            )
                            nc.vector.tensor_tensor(
                                out=z34[:, half * 32 : half * 32 + 32],
                                in0=z34[:, half * 32 : half * 32 + 32],
                                in1=tmp32[:], op=Alu.add,
                            )
                        nc.vector.tensor_tensor(
                            out=z34[:], in0=z34[:], in1=csb["bb34"][:], op=Alu.add
                        )
                        nc.sync.dma_start(
                            z3_out[r0 : r0 + nrow], z34[:nrow, 0:32]
                        )
                        nc.sync.dma_start(
                            z4_out[r0 : r0 + nrow], z34[:nrow, 32:64]
                        )

    nc.compile()
    return nc


# ----------------------------------------------------------------------------
# Entry point
# ----------------------------------------------------------------------------

_CACHE = {}


def _get_program(ns, g, cnts):
    key = (ns, g, tuple(cnts))
    if key not in _CACHE:
        _CACHE[key] = build_program(ns, g, cnts)
    return _CACHE[key]


def kernel(**inputs):
    from concourse import bass_utils

    x = np.ascontiguousarray(np.asarray(inputs["x"], np.float32))
    ei = np.asarray(inputs["edge_index"])
    n = x.shape[0]
    ns = n // R
    g, cnts, per_core = preprocess_graph(ei, n, R)
    consts = make_consts(inputs)
    nc = _get_program(ns, g, cnts)

    in_maps = []
    for r in range(R):
        m = dict(consts)
        m["x_shard"] = x[r * ns : (r + 1) * ns]
        m.update(per_core[r])
        in_maps.append(m)

    res = bass_utils.run_bass_kernel_spmd(nc, in_maps, core_ids=list(range(R)))
    z_mean = np.concatenate([res.results[r]["z3"] for r in range(R)], axis=0)
    z_logstd = np.concatenate([res.results[r]["z4"] for r in range(R)], axis=0)
    return z_mean, z_logstd
